# revision 13
# baseline (speedup 1.0000x reference)
"""CLIPMutationLoss forward on 8 Trainium2 NeuronCores (data-parallel over batch).

Per core b: scores[m, t] = logit_scale * dot(text[b*20+m, t, :], gnn[b, coords[b, t], :])
loss = mean_b( sum_t mask*CE0(scores) / sum_t mask ),  acc = global masked argmax==0 rate.

Device pipeline (per core):
  - text slab host-cast to bf16, laid out [4 chunks, 128 part, 2 d-halves, 20 m, 256 t]:
    one contiguous 2.62 MB DMA per chunk (10.5 MB total), alternating the two HWDGE
    rings (sync / scalar) so chunks stream back-to-back.
  - gnn rows gathered by seq_to_coords with 8 per-token-tile indirect DMAs (fp32,
    [128, 1] offsets — the HW-validated form) on the otherwise-idle GpSimd SWDGE
    path, each PE-transposed into selT[h] = [128 d, 1024 t] (cast to bf16 on the
    PSUM->SBUF copy). Token tile tt unblocks text chunk tt//2, so the first DVE
    mul starts ~9 us in while later gathers still stream.
  - DVE: P[h] = textT_tile * selT_bcast  (bf16 2x mode; d on partitions, (m, t) free)
  - PE: scores = ones-vector matmul reduction over d, both halves accumulated into
    PSUM [128 t-in-tile, 160 = (8 tt) x (20 m)] columns.
  - Epilogue (fp32) in two halves so the first half hides under chunks 2-3:
    scale by logit_scale, log-softmax over m, class-0 CE, argmax==0 (exact:
    correct <=> s0 >= max), masked sums, partition-reduce via ones matmul.
  - Output per core: [loss_masked_sum, correct_masked_sum, mask_sum, 0]; host combines.

bf16 error was validated against the exact seeded inputs: loss rel err ~5e-5 and the
masked accuracy is bit-identical.
"""

import numpy as np

import concourse.bacc as bacc
import concourse.bass as bass
import concourse.tile as tile
from concourse import mybir
from concourse.bass_interp import get_hw_module
from concourse.bass_utils import run_bass_kernel_spmd
from concourse.masks import make_identity

B, N_NODES, D = 8, 2048, 256
T = 1024
M1 = 20  # num_mutations + 1 classes
NCORES = 8
P = 128
NCH = 4            # token chunks per core
CHT = T // NCH     # 256 tokens per chunk
NT = T // P        # 8 token tiles of 128
NH = D // P        # 2 d-halves
F32 = mybir.dt.float32
BF16 = mybir.dt.bfloat16
I32 = mybir.dt.int32
NP_BF16 = mybir.dt.np(BF16)

_NC_CACHE = {}
LAST_RESULTS = None  # test harness reads exec_time_ns off this


def _build_nc():
    nc = bacc.Bacc("TRN2", target_bir_lowering=False, debug=False)
    textT = nc.dram_tensor("textT", [NCH, P, NH, M1, CHT], BF16, kind="ExternalInput").ap()
    gnn = nc.dram_tensor("gnn", [N_NODES, D], F32, kind="ExternalInput").ap()
    coords = nc.dram_tensor("coords", [P, NT], I32, kind="ExternalInput").ap()
    maskf = nc.dram_tensor("maskf", [P, NT], F32, kind="ExternalInput").ap()
    ls = nc.dram_tensor("ls", [P, 1], F32, kind="ExternalInput").ap()
    out = nc.dram_tensor("out", [4, 1], F32, kind="ExternalOutput").ap()

    with (
        tile.TileContext(nc) as tc,
        tc.tile_pool(name="consts", bufs=1) as consts,
        tc.tile_pool(name="selp", bufs=4) as selp,
        tc.tile_pool(name="textp", bufs=3) as textp,
        tc.tile_pool(name="pp", bufs=4) as pp,
        tc.tile_pool(name="soft", bufs=1) as soft,
        tc.tile_pool(name="ps", bufs=1, space="PSUM") as ps,
        tc.tile_pool(name="tps", bufs=2, space="PSUM") as tps,
    ):
        identity = consts.tile([P, P], F32)
        make_identity(nc, identity[:])
        ones_bf = consts.tile([P, 1], BF16)
        nc.vector.memset(ones_bf[:], 1.0)
        ones_f = consts.tile([P, 1], F32)
        nc.vector.memset(ones_f[:], 1.0)
        coords_sb = consts.tile([P, NT], I32)
        nc.scalar.dma_start(out=coords_sb[:], in_=coords[:])
        maskf_sb = consts.tile([P, NT], F32)
        nc.scalar.dma_start(out=maskf_sb[:], in_=maskf[:])
        ls_sb = consts.tile([P, 1], F32)
        nc.scalar.dma_start(out=ls_sb[:], in_=ls[:])

        # Touch Exp/Ln once at kernel start so the ACT table load (~2.7us) hides
        # under the initial DMAs instead of landing on the critical tail.
        dummy = consts.tile([P, 1], F32)
        nc.scalar.activation(out=dummy[:], in_=ones_f[:], func=mybir.ActivationFunctionType.Exp)
        nc.scalar.activation(out=dummy[:], in_=dummy[:], func=mybir.ActivationFunctionType.Ln)

        # Gather selected gnn rows per token tile (fp32, HW-proven indirect form),
        # PE-transpose to [d, t] and cast to bf16 on the PSUM->SBUF copy.
        selT = [consts.tile([P, T], BF16, name=f"selT{h}") for h in range(NH)]
        for tt in range(NT):
            sel_t = selp.tile([P, D], F32, name="sel_t")
            nc.gpsimd.indirect_dma_start(
                out=sel_t[:],
                out_offset=None,
                in_=gnn[:],
                in_offset=bass.IndirectOffsetOnAxis(ap=coords_sb[:, tt : tt + 1], axis=0),
            )
            for h in range(NH):
                tp_ = tps.tile([P, P], F32, name="tp_")
                nc.tensor.transpose(out=tp_[:], in_=sel_t[:, h * P : (h + 1) * P], identity=identity[:])
                nc.scalar.copy(out=selT[h][:, tt * P : (tt + 1) * P], in_=tp_[:])

        # ---- per-token class scores ----
        # columns: col = tt*20 + m  (tt = c*2 + tl, token = tt*128 + p)
        scores_ps = ps.tile([P, NT * M1], F32, name="scores_ps")
        # epilogue tiles (declared up front; written per half)
        scores_sb = soft.tile([P, NT, M1], F32)
        mx = soft.tile([P, NT], F32)
        sub = soft.tile([P, NT, M1], F32)
        expt = soft.tile([P, NT, M1], F32)
        se = soft.tile([P, NT], F32)
        lse = soft.tile([P, NT], F32)
        tmp = soft.tile([P, NT], F32)
        ltok = soft.tile([P, NT], F32)
        corr = soft.tile([P, NT], F32)
        ml = soft.tile([P, NT], F32)
        mc = soft.tile([P, NT], F32)

        def epilogue_half(hv):
            """softmax/CE for token tiles [hv*4, hv*4+4) — runs as soon as the
            corresponding PSUM columns close."""
            ts_ = slice(hv * (NT // 2), (hv + 1) * (NT // 2))
            csl = slice(hv * (NT // 2) * M1, (hv + 1) * (NT // 2) * M1)
            nc.scalar.activation(
                out=scores_sb[:, ts_, :].rearrange("p t m -> p (t m)"),
                in_=scores_ps[:, csl],
                func=mybir.ActivationFunctionType.Copy,
                bias=0.0,
                scale=ls_sb[:, 0:1],
            )
            nc.vector.reduce_max(out=mx[:, ts_], in_=scores_sb[:, ts_, :], axis=mybir.AxisListType.X)
            mxs = mx[:, ts_]
            mx_b = bass.AP(tensor=mx.tensor, offset=mxs.offset, ap=[mxs.ap[0], [1, NT // 2], [0, M1]])
            nc.vector.tensor_tensor(out=sub[:, ts_, :], in0=scores_sb[:, ts_, :], in1=mx_b, op=mybir.AluOpType.subtract)
            nc.scalar.activation(out=expt[:, ts_, :], in_=sub[:, ts_, :], func=mybir.ActivationFunctionType.Exp)
            nc.vector.reduce_sum(out=se[:, ts_], in_=expt[:, ts_, :], axis=mybir.AxisListType.X)
            nc.scalar.activation(out=lse[:, ts_], in_=se[:, ts_], func=mybir.ActivationFunctionType.Ln)
            sc0 = scores_sb[:, ts_, 0:1]
            s0 = bass.AP(tensor=scores_sb.tensor, offset=sc0.offset, ap=[sc0.ap[0], [M1, NT // 2]])
            nc.vector.tensor_add(out=tmp[:, ts_], in0=mx[:, ts_], in1=lse[:, ts_])
            nc.vector.tensor_tensor(out=ltok[:, ts_], in0=tmp[:, ts_], in1=s0, op=mybir.AluOpType.subtract)
            nc.vector.tensor_tensor(out=corr[:, ts_], in0=s0, in1=mx[:, ts_], op=mybir.AluOpType.is_ge)
            nc.vector.tensor_mul(out=ml[:, ts_], in0=ltok[:, ts_], in1=maskf_sb[:, ts_])
            nc.vector.tensor_mul(out=mc[:, ts_], in0=corr[:, ts_], in1=maskf_sb[:, ts_])

        for c in range(NCH):
            tx = textp.tile([P, NH, M1, CHT], BF16, name="tx")
            dma_eng = nc.sync if c % 2 == 0 else nc.scalar
            dma_eng.dma_start(out=tx[:], in_=textT[c])
            ptiles = []
            for h in range(NH):
                pt = pp.tile([P, M1, CHT], BF16, name="pt")
                sl = selT[h][:, c * CHT : (c + 1) * CHT]
                sl_b = bass.AP(tensor=sl.tensor, offset=sl.offset, ap=[sl.ap[0], [0, M1], sl.ap[1]])
                nc.vector.tensor_tensor(out=pt[:], in0=tx[:, h], in1=sl_b, op=mybir.AluOpType.mult)
                ptiles.append(pt)
            for g in range(2 * M1):
                tl, m = divmod(g, M1)
                col = c * 2 * M1 + g
                for h in range(NH):
                    nc.tensor.matmul(
                        out=scores_ps[:, col : col + 1],
                        lhsT=ptiles[h][:, m, tl * P : (tl + 1) * P],
                        rhs=ones_bf[:],
                        start=(h == 0),
                        stop=(h == NH - 1),
                    )
            if c == 1:
                epilogue_half(0)
        epilogue_half(1)

        stats = soft.tile([P, 4], F32)
        nc.vector.memset(stats[:], 0.0)
        nc.vector.reduce_sum(out=stats[:, 0:1], in_=ml[:], axis=mybir.AxisListType.X)
        nc.vector.reduce_sum(out=stats[:, 1:2], in_=mc[:], axis=mybir.AxisListType.X)
        nc.vector.reduce_sum(out=stats[:, 2:3], in_=maskf_sb[:], axis=mybir.AxisListType.X)

        stat_ps = ps.tile([4, 1], F32, name="stat_ps")
        nc.tensor.matmul(out=stat_ps[:], lhsT=stats[:], rhs=ones_f[:], start=True, stop=True)
        out_sb = soft.tile([4, 1], F32)
        nc.scalar.copy(out=out_sb[:], in_=stat_ps[:])
        nc.sync.dma_start(out=out[:], in_=out_sb[:])

    nc.compile()
    nc.m = get_hw_module(nc.m)
    return nc


def get_nc():
    if "nc" not in _NC_CACHE:
        _NC_CACHE["nc"] = _build_nc()
    return _NC_CACHE["nc"]


def make_in_maps(gnn_features, text_features, logit_scale, seq_to_coords, seq_loss_mask):
    in_maps = []
    lsv = np.float32(np.asarray(logit_scale).reshape(-1)[0])
    for b in range(NCORES):
        slab = np.asarray(text_features[b * M1 : (b + 1) * M1], dtype=np.float32)  # [20, 1024, 256]
        tT = slab.transpose(2, 0, 1)                      # [256 d, 20 m, 1024 t]
        tT = tT.reshape(NH, P, M1, NCH, CHT)              # [h, p, m, c, t]
        tT = np.ascontiguousarray(tT.transpose(3, 1, 0, 2, 4)).astype(NP_BF16)  # [c, p, h, m, t]
        in_maps.append(
            {
                "textT": tT,
                "gnn": np.ascontiguousarray(np.asarray(gnn_features[b], dtype=np.float32)),
                "coords": np.ascontiguousarray(
                    np.asarray(seq_to_coords[b]).astype(np.int32).reshape(NT, P).T
                ),
                "maskf": np.ascontiguousarray(
                    np.asarray(seq_loss_mask[b]).astype(np.float32).reshape(NT, P).T
                ),
                "ls": np.full((P, 1), lsv, dtype=np.float32),
            }
        )
    return in_maps


def combine_outputs(results):
    loss = 0.0
    num = 0.0
    den = 0.0
    for r in results:
        o = np.asarray(r["out"], dtype=np.float64).reshape(4)
        loss += o[0] / o[2]
        num += o[1]
        den += o[2]
    loss = np.float32(loss / B)
    acc = np.float32(num / den)
    return np.array(loss, dtype=np.float32), np.array(acc, dtype=np.float32)


def kernel(gnn_features, text_features, logit_scale, seq_to_coords, seq_loss_mask):
    global LAST_RESULTS
    nc = get_nc()
    in_maps = make_in_maps(gnn_features, text_features, logit_scale, seq_to_coords, seq_loss_mask)
    res = run_bass_kernel_spmd(nc, in_maps, core_ids=list(range(NCORES)))
    LAST_RESULTS = res
    return combine_outputs(res.results)


# revision 15
# speedup vs baseline: 1.1047x; 1.1047x over previous
"""CLIPMutationLoss forward on 8 Trainium2 NeuronCores (data-parallel over batch).

Per core b: scores[m, t] = logit_scale * dot(text[b*20+m, t, :], gnn[b, coords[b, t], :])
loss = mean_b( sum_t mask*CE0(scores) / sum_t mask ),  acc = global masked argmax==0 rate.

Device pipeline (per core):
  - text slab host-cast to bf16, laid out [4 chunks, 128 part, 2 d-halves, 20 m, 256 t]:
    one contiguous 2.62 MB DMA per chunk (10.5 MB total), alternating the two HWDGE
    rings (sync / scalar) so chunks stream back-to-back.
  - gnn rows gathered by seq_to_coords with 8 per-token-tile indirect DMAs (fp32,
    [128, 1] offsets — the HW-validated form) on the otherwise-idle GpSimd SWDGE
    path, each PE-transposed into selT[h] = [128 d, 1024 t] (cast to bf16 on the
    PSUM->SBUF copy). Token tile tt unblocks text chunk tt//2, so the first DVE
    mul starts ~9 us in while later gathers still stream.
  - DVE: P[h] = textT_tile * selT_bcast  (bf16 2x mode; d on partitions, (m, t) free)
  - PE: scores = ones-vector matmul reduction over d, both halves accumulated into
    PSUM [128 t-in-tile, 160 = (8 tt) x (20 m)] columns.
  - Epilogue (fp32) in two halves so the first half hides under chunks 2-3:
    scale by logit_scale, log-softmax over m, class-0 CE, argmax==0 (exact:
    correct <=> s0 >= max), masked sums, partition-reduce via ones matmul.
  - Output per core: [loss_masked_sum, correct_masked_sum, mask_sum, 0]; host combines.

bf16 error was validated against the exact seeded inputs: loss rel err ~5e-5 and the
masked accuracy is bit-identical.
"""

import numpy as np

import concourse.bacc as bacc
import concourse.bass as bass
import concourse.tile as tile
from concourse import mybir
from concourse.bass_interp import get_hw_module
from concourse.bass_utils import run_bass_kernel_spmd
from concourse.masks import make_identity

B, N_NODES, D = 8, 2048, 256
T = 1024
M1 = 20  # num_mutations + 1 classes
NCORES = 8
P = 128
NCH = 4            # token chunks per core
CHT = T // NCH     # 256 tokens per chunk
NT = T // P        # 8 token tiles of 128
NH = D // P        # 2 d-halves
F32 = mybir.dt.float32
BF16 = mybir.dt.bfloat16
I32 = mybir.dt.int32
NP_BF16 = mybir.dt.np(BF16)

_NC_CACHE = {}
LAST_RESULTS = None  # test harness reads exec_time_ns off this


def _build_nc():
    nc = bacc.Bacc("TRN2", target_bir_lowering=False, debug=False)
    textT = nc.dram_tensor("textT", [NCH, P, NH, M1, CHT], BF16, kind="ExternalInput").ap()
    gnn = nc.dram_tensor("gnn", [N_NODES, D], F32, kind="ExternalInput").ap()
    coords = nc.dram_tensor("coords", [P, NT], I32, kind="ExternalInput").ap()
    maskf = nc.dram_tensor("maskf", [P, NT], F32, kind="ExternalInput").ap()
    ls = nc.dram_tensor("ls", [P, 1], F32, kind="ExternalInput").ap()
    out = nc.dram_tensor("out", [4, 1], F32, kind="ExternalOutput").ap()

    with (
        tile.TileContext(nc) as tc,
        tc.tile_pool(name="consts", bufs=1) as consts,
        tc.tile_pool(name="selp", bufs=8) as selp,
        tc.tile_pool(name="textp", bufs=3) as textp,
        tc.tile_pool(name="pp", bufs=4) as pp,
        tc.tile_pool(name="soft", bufs=1) as soft,
        tc.tile_pool(name="ps", bufs=1, space="PSUM") as ps,
        tc.tile_pool(name="tps", bufs=2, space="PSUM") as tps,
    ):
        identity = consts.tile([P, P], F32)
        make_identity(nc, identity[:])
        ones_bf = consts.tile([P, 1], BF16)
        nc.vector.memset(ones_bf[:], 1.0)
        ones_f = consts.tile([P, 1], F32)
        nc.vector.memset(ones_f[:], 1.0)
        # coords ride the gpsimd SWDGE queue: it feeds the gathers on that same
        # engine, so the scheduler cannot park it behind a 2.6MB text DMA the
        # way the HWDGE rings did (that stalled the whole gather head ~13us).
        coords_sb = consts.tile([P, NT], I32)
        nc.gpsimd.dma_start(out=coords_sb[:], in_=coords[:])
        maskf_sb = consts.tile([P, NT], F32)
        nc.scalar.dma_start(out=maskf_sb[:], in_=maskf[:])
        ls_sb = consts.tile([P, 1], F32)
        nc.scalar.dma_start(out=ls_sb[:], in_=ls[:])

        # Touch Exp/Ln once at kernel start so the ACT table load (~2.7us) hides
        # under the initial DMAs instead of landing on the critical tail.
        dummy = consts.tile([P, 1], F32)
        nc.scalar.activation(out=dummy[:], in_=ones_f[:], func=mybir.ActivationFunctionType.Exp)
        nc.scalar.activation(out=dummy[:], in_=dummy[:], func=mybir.ActivationFunctionType.Ln)

        # Gather selected gnn rows per token tile (fp32, HW-proven indirect form),
        # PE-transpose to [d, t] and cast to bf16 on the PSUM->SBUF copy.
        selT = [consts.tile([P, T], BF16, name=f"selT{h}") for h in range(NH)]
        for tt in range(NT):
            sel_t = selp.tile([P, D], F32, name="sel_t")
            nc.gpsimd.indirect_dma_start(
                out=sel_t[:],
                out_offset=None,
                in_=gnn[:],
                in_offset=bass.IndirectOffsetOnAxis(ap=coords_sb[:, tt : tt + 1], axis=0),
            )
            for h in range(NH):
                tp_ = tps.tile([P, P], F32, name="tp_")
                nc.tensor.transpose(out=tp_[:], in_=sel_t[:, h * P : (h + 1) * P], identity=identity[:])
                nc.scalar.copy(out=selT[h][:, tt * P : (tt + 1) * P], in_=tp_[:])

        # ---- per-token class scores ----
        # columns: col = tt*20 + m  (tt = c*2 + tl, token = tt*128 + p)
        scores_ps = ps.tile([P, NT * M1], F32, name="scores_ps")
        # epilogue tiles (declared up front; written per half)
        scores_sb = soft.tile([P, NT, M1], F32)
        mx = soft.tile([P, NT], F32)
        sub = soft.tile([P, NT, M1], F32)
        expt = soft.tile([P, NT, M1], F32)
        se = soft.tile([P, NT], F32)
        lse = soft.tile([P, NT], F32)
        tmp = soft.tile([P, NT], F32)
        ltok = soft.tile([P, NT], F32)
        corr = soft.tile([P, NT], F32)
        ml = soft.tile([P, NT], F32)
        mc = soft.tile([P, NT], F32)

        def epilogue_half(hv):
            """softmax/CE for token tiles [hv*4, hv*4+4) — runs as soon as the
            corresponding PSUM columns close."""
            ts_ = slice(hv * (NT // 2), (hv + 1) * (NT // 2))
            csl = slice(hv * (NT // 2) * M1, (hv + 1) * (NT // 2) * M1)
            nc.scalar.activation(
                out=scores_sb[:, ts_, :].rearrange("p t m -> p (t m)"),
                in_=scores_ps[:, csl],
                func=mybir.ActivationFunctionType.Copy,
                bias=0.0,
                scale=ls_sb[:, 0:1],
            )
            nc.vector.reduce_max(out=mx[:, ts_], in_=scores_sb[:, ts_, :], axis=mybir.AxisListType.X)
            mxs = mx[:, ts_]
            mx_b = bass.AP(tensor=mx.tensor, offset=mxs.offset, ap=[mxs.ap[0], [1, NT // 2], [0, M1]])
            nc.vector.tensor_tensor(out=sub[:, ts_, :], in0=scores_sb[:, ts_, :], in1=mx_b, op=mybir.AluOpType.subtract)
            nc.scalar.activation(out=expt[:, ts_, :], in_=sub[:, ts_, :], func=mybir.ActivationFunctionType.Exp)
            nc.vector.reduce_sum(out=se[:, ts_], in_=expt[:, ts_, :], axis=mybir.AxisListType.X)
            nc.scalar.activation(out=lse[:, ts_], in_=se[:, ts_], func=mybir.ActivationFunctionType.Ln)
            sc0 = scores_sb[:, ts_, 0:1]
            s0 = bass.AP(tensor=scores_sb.tensor, offset=sc0.offset, ap=[sc0.ap[0], [M1, NT // 2]])
            nc.vector.tensor_add(out=tmp[:, ts_], in0=mx[:, ts_], in1=lse[:, ts_])
            nc.vector.tensor_tensor(out=ltok[:, ts_], in0=tmp[:, ts_], in1=s0, op=mybir.AluOpType.subtract)
            nc.vector.tensor_tensor(out=corr[:, ts_], in0=s0, in1=mx[:, ts_], op=mybir.AluOpType.is_ge)
            nc.vector.tensor_mul(out=ml[:, ts_], in0=ltok[:, ts_], in1=maskf_sb[:, ts_])
            nc.vector.tensor_mul(out=mc[:, ts_], in0=corr[:, ts_], in1=maskf_sb[:, ts_])

        for c in range(NCH):
            tx = textp.tile([P, NH, M1, CHT], BF16, name="tx")
            dma_eng = nc.sync if c % 2 == 0 else nc.scalar
            dma_eng.dma_start(out=tx[:], in_=textT[c])
            ptiles = []
            for h in range(NH):
                pt = pp.tile([P, M1, CHT], BF16, name="pt")
                sl = selT[h][:, c * CHT : (c + 1) * CHT]
                sl_b = bass.AP(tensor=sl.tensor, offset=sl.offset, ap=[sl.ap[0], [0, M1], sl.ap[1]])
                nc.vector.tensor_tensor(out=pt[:], in0=tx[:, h], in1=sl_b, op=mybir.AluOpType.mult)
                ptiles.append(pt)
            for g in range(2 * M1):
                tl, m = divmod(g, M1)
                col = c * 2 * M1 + g
                for h in range(NH):
                    nc.tensor.matmul(
                        out=scores_ps[:, col : col + 1],
                        lhsT=ptiles[h][:, m, tl * P : (tl + 1) * P],
                        rhs=ones_bf[:],
                        start=(h == 0),
                        stop=(h == NH - 1),
                    )
            if c == 1:
                epilogue_half(0)
        epilogue_half(1)

        stats = soft.tile([P, 4], F32)
        nc.vector.memset(stats[:], 0.0)
        nc.vector.reduce_sum(out=stats[:, 0:1], in_=ml[:], axis=mybir.AxisListType.X)
        nc.vector.reduce_sum(out=stats[:, 1:2], in_=mc[:], axis=mybir.AxisListType.X)
        nc.vector.reduce_sum(out=stats[:, 2:3], in_=maskf_sb[:], axis=mybir.AxisListType.X)

        stat_ps = ps.tile([4, 1], F32, name="stat_ps")
        nc.tensor.matmul(out=stat_ps[:], lhsT=stats[:], rhs=ones_f[:], start=True, stop=True)
        out_sb = soft.tile([4, 1], F32)
        nc.scalar.copy(out=out_sb[:], in_=stat_ps[:])
        nc.sync.dma_start(out=out[:], in_=out_sb[:])

    nc.compile()
    nc.m = get_hw_module(nc.m)
    return nc


def get_nc():
    if "nc" not in _NC_CACHE:
        _NC_CACHE["nc"] = _build_nc()
    return _NC_CACHE["nc"]


def make_in_maps(gnn_features, text_features, logit_scale, seq_to_coords, seq_loss_mask):
    in_maps = []
    lsv = np.float32(np.asarray(logit_scale).reshape(-1)[0])
    for b in range(NCORES):
        slab = np.asarray(text_features[b * M1 : (b + 1) * M1], dtype=np.float32)  # [20, 1024, 256]
        tT = slab.transpose(2, 0, 1)                      # [256 d, 20 m, 1024 t]
        tT = tT.reshape(NH, P, M1, NCH, CHT)              # [h, p, m, c, t]
        tT = np.ascontiguousarray(tT.transpose(3, 1, 0, 2, 4)).astype(NP_BF16)  # [c, p, h, m, t]
        in_maps.append(
            {
                "textT": tT,
                "gnn": np.ascontiguousarray(np.asarray(gnn_features[b], dtype=np.float32)),
                "coords": np.ascontiguousarray(
                    np.asarray(seq_to_coords[b]).astype(np.int32).reshape(NT, P).T
                ),
                "maskf": np.ascontiguousarray(
                    np.asarray(seq_loss_mask[b]).astype(np.float32).reshape(NT, P).T
                ),
                "ls": np.full((P, 1), lsv, dtype=np.float32),
            }
        )
    return in_maps


def combine_outputs(results):
    loss = 0.0
    num = 0.0
    den = 0.0
    for r in results:
        o = np.asarray(r["out"], dtype=np.float64).reshape(4)
        loss += o[0] / o[2]
        num += o[1]
        den += o[2]
    loss = np.float32(loss / B)
    acc = np.float32(num / den)
    return np.array(loss, dtype=np.float32), np.array(acc, dtype=np.float32)


def kernel(gnn_features, text_features, logit_scale, seq_to_coords, seq_loss_mask):
    global LAST_RESULTS
    nc = get_nc()
    in_maps = make_in_maps(gnn_features, text_features, logit_scale, seq_to_coords, seq_loss_mask)
    res = run_bass_kernel_spmd(nc, in_maps, core_ids=list(range(NCORES)))
    LAST_RESULTS = res
    return combine_outputs(res.results)
